# revision 6
# baseline (speedup 1.0000x reference)
"""Multi-head attention (B=2, S=2048, H=16, D=64) on 8 Trainium2 NeuronCores.

Sharding: head-parallel tensor parallelism. Core c owns heads {2c, 2c+1}
(a 128-dim slice of the model dim): column-parallel QKV projections and
local causal attention for its 2 heads, then an AllToAll of normalized
bf16 context vectors (1 MiB/core per half-batch) pipelined behind the
attention loop; each core runs the full-width Wo projection for its own
disjoint 128-token slices and writes final output rows directly.

Key structure (v2, rebuilt around the measured baseline trace):
- Weights load before x; x streams in four 2 MB contiguous chunks with a
  host-side [p, kc, t] layout, so the first projection matmul issues a
  few microseconds in instead of ~50 us.
- Scores run as two concurrent K=64 matmuls (head 0 on PE rows 0-63,
  head 1 on rows 64-127 via tile_position row tiling) into adjacent
  PSUM banks; one Exp activation covers both heads through a strided
  PSUM access pattern, halving the per-op ACT overhead.
- Attention-times-V keeps V as the stationary operand ([keys, 64+ones])
  and streams the exp tile, so each key block costs one weight load per
  head and the context lands directly in [dims, tokens] layout -- no
  PE transposes anywhere in the kernel.
- The softmax denominator rides a ones-column in the V stationary; the
  per-query reciprocal row is broadcast across partitions with a K=1
  matmul and folded in with one vector multiply per (head, 512 queries).
- V projections are computed directly transposed (x chunk stationary,
  Wv moving), packed 8 token-blocks per PSUM bank pair.
- Batch-1 projections are emitted between batch-0 attention groups so
  the PE stays fed while the scalar engine (the attention-phase
  bottleneck) churns through exp.
"""

import sys

sys.path.insert(0, "/opt/trn_rl_repo")

import ml_dtypes
import numpy as np

import concourse.bass as bass
import concourse.tile as tile
from concourse import bacc, mybir
from concourse.bass_utils import run_bass_kernel_spmd

N_CORES = 8
B, S, H, D = 2, 2048, 16, 64
E = H * D            # 1024
T = B * S            # 4096 tokens
DPC = 128            # dims (2 heads) per core
NKC = E // 128       # 8 contraction chunks for the projections
NTT = T // 512       # 8 token tiles of 512
NTB = T // 128       # 32 token blocks of 128
SB = S // 128        # 16 key blocks per batch
PH = S // 2 // N_CORES  # 128 tokens per core per half-batch

F32 = mybir.dt.float32
BF16 = mybir.dt.bfloat16
AFT = mybir.ActivationFunctionType


def build_program():
    nc = bacc.Bacc("TRN2", target_bir_lowering=False, debug=False,
                   num_devices=N_CORES)

    # host pre-arranged layouts (see kernel()):
    #   xh[p, kc, t]  = x^T[kc*128+p, t]          (bf16)
    #   w*h[p, kc, d] = W*^T[kc*128+p, d]         (bf16)
    xh = nc.dram_tensor("xh", [128, NKC, T], BF16, kind="ExternalInput").ap()
    wqh = nc.dram_tensor("wqh", [128, NKC, DPC], BF16, kind="ExternalInput").ap()
    wkh = nc.dram_tensor("wkh", [128, NKC, DPC], BF16, kind="ExternalInput").ap()
    wvh = nc.dram_tensor("wvh", [128, NKC, DPC], BF16, kind="ExternalInput").ap()
    woh = nc.dram_tensor("woh", [128, NKC, E], BF16, kind="ExternalInput").ap()
    bq = nc.dram_tensor("bq", [DPC, 1], F32, kind="ExternalInput").ap()
    bk = nc.dram_tensor("bk", [DPC, 1], F32, kind="ExternalInput").ap()
    bvv = nc.dram_tensor("bvv", [132], F32, kind="ExternalInput").ap()
    bo = nc.dram_tensor("bo", [E], F32, kind="ExternalInput").ap()
    # [128, 128] lower-triangular keep-mask (k_local <= q_local)
    tri = nc.dram_tensor("tri", [128, 128], BF16, kind="ExternalInput").ap()
    out = nc.dram_tensor("out", [T // N_CORES, E], F32, kind="ExternalOutput").ap()

    with tile.TileContext(nc) as tc:
        with (
            tc.tile_pool(name="consts", bufs=1) as consts,
            tc.tile_pool(name="state", bufs=1) as state,
            tc.tile_pool(name="ep", bufs=3) as ep,
            tc.tile_pool(name="rp", bufs=2) as rp,
            tc.tile_pool(name="op", bufs=2) as op,
            tc.tile_pool(name="ps_s", bufs=2, space="PSUM") as ps_s,
            tc.tile_pool(name="ps_c", bufs=3, space="PSUM") as ps_c,
            tc.tile_pool(name="ps_w", bufs=1, space="PSUM") as ps_w,
            tc.tile_pool(name="dram", bufs=1, space="DRAM") as dram,
        ):
            # ---- constants (queued ahead of x so compute can start) -------
            bq_sb = consts.tile([128, 1], F32)
            bk_sb = consts.tile([128, 1], F32)
            nc.sync.dma_start(out=bq_sb[:], in_=bq[:])
            nc.sync.dma_start(out=bk_sb[:], in_=bk[:])
            bv_bc = consts.tile([128, 132], F32)
            nc.sync.dma_start(
                out=bv_bc[:],
                in_=bass.AP(tensor=bvv.tensor, offset=bvv.offset,
                            ap=[[0, 128], [1, 132]]))
            bo_bc = consts.tile([128, E], F32)
            nc.sync.dma_start(
                out=bo_bc[:],
                in_=bass.AP(tensor=bo.tensor, offset=bo.offset,
                            ap=[[0, 128], [1, E]]))
            tri_sb = consts.tile([128, 128], BF16)
            nc.sync.dma_start(out=tri_sb[:], in_=tri[:])
            ones65 = consts.tile([1, 65], BF16)
            nc.vector.memset(ones65[:], 1.0)

            wq_sb = consts.tile([128, NKC, DPC], BF16)
            wk_sb = consts.tile([128, NKC, DPC], BF16)
            wv_sb = consts.tile([128, NKC, DPC], BF16)
            nc.sync.dma_start(out=wq_sb[:], in_=wqh[:])
            nc.sync.dma_start(out=wk_sb[:], in_=wkh[:])
            nc.sync.dma_start(out=wv_sb[:], in_=wvh[:])

            # ---- x: four 2 MB contiguous-run chunks, token-major ----------
            x_sb = state.tile([128, NKC, T], BF16)
            for tt2 in range(4):
                ts = slice(tt2 * 1024, (tt2 + 1) * 1024)
                nc.sync.dma_start(out=x_sb[:, :, ts], in_=xh[:, :, ts])
            wo_sb = consts.tile([128, NKC, E], BF16)
            nc.sync.dma_start(out=wo_sb[:], in_=woh[:])

            # ---- persistent activations ----------------------------------
            qT_sb = state.tile([128, T], BF16)   # [2-head dims, tokens]
            kT_sb = state.tile([128, T], BF16)
            # [tok, h0 v(64), ones, pad, h1 v(64), ones, pad]
            vN_sb = state.tile([128, NTB, 132], BF16)
            ctxT_sb = state.tile([128, T], BF16)  # normalized ctx, [dims, tok]

            nc.vector.memset(vN_sb[:, :, 64:65], 1.0)
            nc.vector.memset(vN_sb[:, :, 65:66], 0.0)
            nc.vector.memset(vN_sb[:, :, 130:131], 1.0)
            nc.vector.memset(vN_sb[:, :, 131:132], 0.0)

            # ---- phase A unit: Q/K projections for one 512-token tile ----
            def emit_qk(tt):
                ts = slice(tt * 512, (tt + 1) * 512)
                ps = ps_s.tile([128, 2, 512], F32, tag="s", name="qk_ps")
                for kc in range(NKC):
                    nc.tensor.matmul(ps[:, 0, :], wq_sb[:, kc, :],
                                     x_sb[:, kc, ts],
                                     start=(kc == 0), stop=(kc == NKC - 1))
                for kc in range(NKC):
                    nc.tensor.matmul(ps[:, 1, :], wk_sb[:, kc, :],
                                     x_sb[:, kc, ts],
                                     start=(kc == 0), stop=(kc == NKC - 1))
                nc.vector.tensor_scalar_add(qT_sb[:, ts], ps[:, 0, :],
                                            bq_sb[:])
                nc.vector.tensor_scalar_add(kT_sb[:, ts], ps[:, 1, :],
                                            bk_sb[:])

            # ---- phase A unit: direct-transposed V for 8 token blocks ----
            def emit_v8(tb0):
                ps = ps_s.tile([128, 2, 512], F32, tag="s", name="v_ps")
                for tb8 in range(8):
                    tb = tb0 + tb8
                    tc_ = slice(tb * 128, (tb + 1) * 128)
                    c = (tb8 % 4) * 128
                    for kc in range(NKC):
                        nc.tensor.matmul(
                            ps[:, tb8 // 4, c:c + 128],
                            x_sb[:, kc, tc_], wv_sb[:, kc, :],
                            start=(kc == 0 and tb8 % 4 == 0),
                            stop=(kc == NKC - 1 and tb8 % 4 == 3),
                            skip_group_check=True)
                for tb8 in range(8):
                    tb = tb0 + tb8
                    c = (tb8 % 4) * 128
                    nc.vector.tensor_add(vN_sb[:, tb, 0:64],
                                         ps[:, tb8 // 4, c:c + 64],
                                         bv_bc[:, 0:64])
                    nc.vector.tensor_add(vN_sb[:, tb, 66:130],
                                         ps[:, tb8 // 4, c + 64:c + 128],
                                         bv_bc[:, 66:130])

            # ---- attention for one (batch, 512-query group) --------------
            def emit_scores(b, qt, kb):
                t0 = b * S
                q0 = t0 + qt * 512
                c0 = max(kb - 4 * qt, 0) * 128
                ks = slice(t0 + kb * 128, t0 + (kb + 1) * 128)
                s_ps = ps_s.tile([128, 2, 512], F32, tag="s", name="s_ps")
                nc.tensor.matmul(s_ps[:, 0, c0:512],
                                 kT_sb[0:64, ks], qT_sb[0:64, q0 + c0:q0 + 512],
                                 start=True, stop=True)
                nc.tensor.matmul(s_ps[:, 1, c0:512],
                                 kT_sb[64:128, ks],
                                 qT_sb[64:128, q0 + c0:q0 + 512],
                                 start=True, stop=True)
                return s_ps

            def emit_attn_qt(b, qt, fillers):
                t0 = b * S
                q0 = t0 + qt * 512
                nkb = 4 * qt + 4
                ctx_ps = [ps_c.tile([65, 512], F32, tag="c", name=f"ctx{h}")
                          for h in range(2)]
                s_tiles = {0: emit_scores(b, qt, 0)}
                for kb in range(nkb):
                    m = kb - 4 * qt
                    c0 = max(m, 0) * 128
                    if kb + 1 < nkb:
                        s_tiles[kb + 1] = emit_scores(b, qt, kb + 1)
                    s_ps = s_tiles.pop(kb)
                    e_sb = ep.tile([128, 2, 512], BF16, tag="e", name="e_sb")
                    nc.scalar.activation(e_sb[:, :, c0:512],
                                         s_ps[:, :, c0:512],
                                         AFT.Exp, scale=0.125)
                    if m >= 0:  # triangular block on the diagonal
                        for h in range(2):
                            nc.vector.tensor_mul(
                                e_sb[:, h, c0:c0 + 128],
                                e_sb[:, h, c0:c0 + 128], tri_sb[:])
                    for h in range(2):
                        nc.tensor.matmul(
                            ctx_ps[h][0:65, c0:512],
                            vN_sb[:, b * SB + kb, 66 * h:66 * h + 65],
                            e_sb[:, h, c0:512],
                            start=(kb == 0), stop=(kb == nkb - 1),
                            skip_group_check=True)
                    if fillers and kb % 4 == 3:
                        fillers.pop(0)()
                # normalize: ctx rows 0-63, denominator row 64
                for h in range(2):
                    recip = rp.tile([1, 512], BF16, tag="r", name="recip")
                    with nc.allow_low_precision("bf16 softmax denom recip"):
                        nc.vector.reciprocal(recip[:], ctx_ps[h][64:65, :])
                    bc_ps = ps_c.tile([65, 512], F32, tag="c", name="bc_ps")
                    nc.tensor.matmul(bc_ps[:], ones65[:], recip[:],
                                     start=True, stop=True)
                    # TensorTensor may read at most one PSUM operand, so
                    # stage the broadcast reciprocal through SBUF.
                    bc_sb = rp.tile([64, 512], BF16, tag="bc", name="bc_sb")
                    nc.vector.tensor_copy(bc_sb[:], bc_ps[0:64, :])
                    nc.vector.tensor_mul(
                        ctxT_sb[64 * h:64 * h + 64, q0:q0 + 512],
                        ctx_ps[h][0:64, :], bc_sb[:])
                while fillers:  # flush anything not consumed mid-loop
                    fillers.pop(0)()

            # ---- AllToAll + local full-width output projection -----------
            def emit_half_a2a(b, hf):
                base = b * S + hf * (S // 2)
                ctxd = dram.tile([N_CORES, 128, PH], BF16, tag="ctxd",
                                 name="ctxd", bufs=4)
                for j in range(N_CORES):
                    nc.sync.dma_start(
                        out=ctxd[j],
                        in_=ctxT_sb[:, base + j * PH:base + (j + 1) * PH])
                recv = dram.tile([N_CORES, 128, PH], BF16, tag="recv",
                                 name="recv", bufs=4)
                nc.gpsimd.collective_compute(
                    "AllToAll",
                    mybir.AluOpType.bypass,
                    replica_groups=[list(range(N_CORES))],
                    ins=[ctxd.opt()],
                    outs=[recv.opt()],
                )
                return recv

            def emit_half_proj(b, hf, recv):
                cg_sb = op.tile([128, NKC, PH], BF16, tag="cg", name="cg_sb")
                for j in range(N_CORES):
                    nc.sync.dma_start(out=cg_sb[:, j, :], in_=recv[j])
                o_sb = op.tile([PH, E], F32, tag="o", name="o_sb")
                for et in range(2):
                    ps = ps_w.tile([128, 512], F32, tag="w", name="w_ps")
                    for kc in range(NKC):
                        nc.tensor.matmul(
                            ps[0:PH, :],
                            cg_sb[:, kc, :],
                            wo_sb[:, kc, et * 512:(et + 1) * 512],
                            start=(kc == 0), stop=(kc == NKC - 1))
                    nc.vector.tensor_add(
                        o_sb[:, et * 512:(et + 1) * 512], ps[0:PH, :],
                        bo_bc[0:PH, et * 512:(et + 1) * 512])
                r0 = (b * 2 + hf) * PH
                nc.sync.dma_start(out=out[r0:r0 + PH, :], in_=o_sb[:])

            # ---- schedule -------------------------------------------------
            # Phase A for batch 0 up front; batch 1's A-units and the
            # Wo projections slot in as PE fillers between batch-0 qt
            # groups (the attention phase is ACT-bound).
            for tt in range(4):
                emit_qk(tt)
                if tt % 2 == 1:
                    emit_v8((tt - 1) * 4)

            pending = []

            def mk_fill(fn, *a):
                return lambda: fn(*a)

            for b in range(B):
                for qt in range(4):
                    fillers = []
                    if b == 0:
                        tt = 4 + qt
                        fillers.append(mk_fill(emit_qk, tt))
                        if qt % 2 == 1:
                            fillers.append(mk_fill(emit_v8, (tt - 1) * 4))
                    if pending and (qt % 2 == 0):
                        fillers.append(mk_fill(emit_half_proj,
                                               *pending.pop(0)))
                    emit_attn_qt(b, qt, fillers)
                    if qt == 1 or qt == 3:
                        pending.append((b, qt // 2, emit_half_a2a(b, qt // 2)))

            while pending:
                emit_half_proj(*pending.pop(0))

    nc.compile()
    return nc


_NC = None


def _get_program():
    global _NC
    if _NC is None:
        _NC = build_program()
    return _NC


def _bf(a):
    return np.ascontiguousarray(a).astype(ml_dtypes.bfloat16)


def _pkc(w):
    """[E, d] -> [p(128), kc(8), d] host layout."""
    d = w.shape[1]
    return np.ascontiguousarray(
        w.reshape(NKC, 128, d).transpose(1, 0, 2))


def kernel(x, Wq, bq, Wk, bk, Wv, bv, Wo, bo, _trace=False, _trace_kwargs=None):
    x = np.asarray(x, np.float32)
    Wq, Wk, Wv, Wo = (np.asarray(w, np.float32) for w in (Wq, Wk, Wv, Wo))
    bq, bk, bv, bo = (np.asarray(v, np.float32) for v in (bq, bk, bv, bo))

    xh = _bf(_pkc(x.reshape(T, E).T))
    i = np.arange(128)
    tri = _bf((i[:, None] <= i[None, :]).astype(np.float32))
    woh = _bf(_pkc(Wo.T))

    in_maps = []
    for c in range(N_CORES):
        sl = slice(c * DPC, (c + 1) * DPC)
        bvv = np.zeros(132, np.float32)
        bvv[0:64] = bv[sl][0:64]
        bvv[66:130] = bv[sl][64:128]
        in_maps.append({
            "xh": xh,
            "wqh": _bf(_pkc(Wq[sl, :].T)),
            "wkh": _bf(_pkc(Wk[sl, :].T)),
            "wvh": _bf(_pkc(Wv[sl, :].T)),
            "woh": woh,
            "bq": bq[sl].reshape(DPC, 1).copy(),
            "bk": bk[sl].reshape(DPC, 1).copy(),
            "bvv": bvv,
            "bo": bo,
            "tri": tri,
        })

    nc = _get_program()
    res = run_bass_kernel_spmd(nc, in_maps, list(range(N_CORES)),
                               trace=_trace, **(_trace_kwargs or {}))
    # out[c] rows are [batch, half, 128]: row (b, hf, r) holds global
    # token b*2048 + hf*1024 + c*128 + r.
    stacked = np.stack([res.results[i]["out"].reshape(B, 2, 128, E)
                        for i in range(N_CORES)], axis=2)
    full = stacked.reshape(T, E)
    if _trace:
        return full.reshape(B, S, E), res
    return full.reshape(B, S, E)


# revision 14
# speedup vs baseline: 1.0783x; 1.0783x over previous
"""Multi-head attention (B=2, S=2048, H=16, D=64) on 8 Trainium2 NeuronCores.

Sharding: head-parallel tensor parallelism. Core c owns heads {2c, 2c+1}
(a 128-dim slice of the model dim): column-parallel QKV projections and
local causal attention for its 2 heads, then an AllToAll of normalized
bf16 context vectors (1 MiB/core per half-batch) pipelined behind the
attention loop; each core runs the full-width Wo projection for its own
disjoint 128-token slices and writes final output rows directly.

Key structure (v2, rebuilt around the measured baseline trace):
- Weights load before x; x streams in four 2 MB contiguous chunks with a
  host-side [p, kc, t] layout, so the first projection matmul issues a
  few microseconds in instead of ~50 us.
- Scores run as two concurrent K=64 matmuls (head 0 on PE rows 0-63,
  head 1 on rows 64-127 via tile_position row tiling) into adjacent
  PSUM banks; one Exp activation covers both heads through a strided
  PSUM access pattern, halving the per-op ACT overhead.
- Attention-times-V keeps V as the stationary operand ([keys, 64+ones])
  and streams the exp tile, so each key block costs one weight load per
  head and the context lands directly in [dims, tokens] layout -- no
  PE transposes anywhere in the kernel.
- The softmax denominator rides a ones-column in the V stationary; the
  per-query reciprocal row is broadcast across partitions with a K=1
  matmul and folded in with one vector multiply per (head, 512 queries).
- V projections are computed directly transposed (x chunk stationary,
  Wv moving), packed 8 token-blocks per PSUM bank pair.
- Batch-1 projections are emitted between batch-0 attention groups so
  the PE stays fed while the scalar engine (the attention-phase
  bottleneck) churns through exp.
"""

import sys

sys.path.insert(0, "/opt/trn_rl_repo")

import ml_dtypes
import numpy as np

import concourse.bass as bass
import concourse.tile as tile
from concourse import bacc, mybir
from concourse.bass_utils import run_bass_kernel_spmd

N_CORES = 8
B, S, H, D = 2, 2048, 16, 64
E = H * D            # 1024
T = B * S            # 4096 tokens
DPC = 128            # dims (2 heads) per core
NKC = E // 128       # 8 contraction chunks for the projections
NTT = T // 512       # 8 token tiles of 512
NTB = T // 128       # 32 token blocks of 128
SB = S // 128        # 16 key blocks per batch
PH = S // 2 // N_CORES  # 128 tokens per core per half-batch

F32 = mybir.dt.float32
BF16 = mybir.dt.bfloat16
AFT = mybir.ActivationFunctionType


def build_program():
    nc = bacc.Bacc("TRN2", target_bir_lowering=False, debug=False,
                   num_devices=N_CORES)

    # host pre-arranged layouts (see kernel()):
    #   xh[p, kc, t]  = x^T[kc*128+p, t]          (bf16)
    #   w*h[p, kc, d] = W*^T[kc*128+p, d]         (bf16)
    xh = nc.dram_tensor("xh", [128, NKC, T], BF16, kind="ExternalInput").ap()
    wqh = nc.dram_tensor("wqh", [128, NKC, DPC], BF16, kind="ExternalInput").ap()
    wkh = nc.dram_tensor("wkh", [128, NKC, DPC], BF16, kind="ExternalInput").ap()
    wvh = nc.dram_tensor("wvh", [128, NKC, DPC], BF16, kind="ExternalInput").ap()
    woh = nc.dram_tensor("woh", [128, NKC, E], BF16, kind="ExternalInput").ap()
    bq = nc.dram_tensor("bq", [DPC, 1], F32, kind="ExternalInput").ap()
    bk = nc.dram_tensor("bk", [DPC, 1], F32, kind="ExternalInput").ap()
    bvv = nc.dram_tensor("bvv", [132], F32, kind="ExternalInput").ap()
    bo = nc.dram_tensor("bo", [E], F32, kind="ExternalInput").ap()
    # [128, 128] lower-triangular keep-mask (k_local <= q_local)
    tri = nc.dram_tensor("tri", [128, 128], BF16, kind="ExternalInput").ap()
    out = nc.dram_tensor("out", [T // N_CORES, E], F32, kind="ExternalOutput").ap()

    with tile.TileContext(nc) as tc:
        with (
            tc.tile_pool(name="consts", bufs=1) as consts,
            tc.tile_pool(name="state", bufs=1) as state,
            tc.tile_pool(name="ep", bufs=3) as ep,
            tc.tile_pool(name="rp", bufs=2) as rp,
            tc.tile_pool(name="op", bufs=2) as op,
            tc.tile_pool(name="ps_s", bufs=2, space="PSUM") as ps_s,
            tc.tile_pool(name="ps_c", bufs=3, space="PSUM") as ps_c,
            tc.tile_pool(name="ps_w", bufs=1, space="PSUM") as ps_w,
            tc.tile_pool(name="dram", bufs=1, space="DRAM") as dram,
        ):
            # ---- constants (queued ahead of x so compute can start) -------
            bq_sb = consts.tile([128, 1], F32)
            bk_sb = consts.tile([128, 1], F32)
            nc.sync.dma_start(out=bq_sb[:], in_=bq[:])
            nc.sync.dma_start(out=bk_sb[:], in_=bk[:])
            bv_bc = consts.tile([128, 132], F32)
            nc.sync.dma_start(
                out=bv_bc[:],
                in_=bass.AP(tensor=bvv.tensor, offset=bvv.offset,
                            ap=[[0, 128], [1, 132]]))
            bo_bc = consts.tile([128, E], F32)
            nc.sync.dma_start(
                out=bo_bc[:],
                in_=bass.AP(tensor=bo.tensor, offset=bo.offset,
                            ap=[[0, 128], [1, E]]))
            tri_sb = consts.tile([128, 128], BF16)
            nc.sync.dma_start(out=tri_sb[:], in_=tri[:])
            ones65 = consts.tile([1, 65], BF16)
            nc.vector.memset(ones65[:], 1.0)

            wq_sb = consts.tile([128, NKC, DPC], BF16)
            wk_sb = consts.tile([128, NKC, DPC], BF16)
            wv_sb = consts.tile([128, NKC, DPC], BF16)
            nc.sync.dma_start(out=wq_sb[:], in_=wqh[:])
            nc.sync.dma_start(out=wk_sb[:], in_=wkh[:])
            nc.sync.dma_start(out=wv_sb[:], in_=wvh[:])

            # ---- x: four 2 MB contiguous-run chunks, token-major, in
            # separate tiles (so the first projection only waits on its own
            # chunk) spread over both HWDGE queues (sync + scalar) ----------
            x_c = [state.tile([128, NKC, 1024], BF16, name=f"x_c{i}")
                   for i in range(4)]
            for tt2 in range(4):
                ts = slice(tt2 * 1024, (tt2 + 1) * 1024)
                eng = nc.sync if tt2 % 2 == 0 else nc.scalar
                eng.dma_start(out=x_c[tt2][:], in_=xh[:, :, ts])

            def x_sl(kc, t0_, n):
                # slice of x^T[kc] covering tokens [t0_, t0_+n) (chunk-local)
                o = t0_ % 1024
                return x_c[t0_ // 1024][:, kc, o:o + n]

            wo_sb = consts.tile([128, NKC, E], BF16)
            nc.scalar.dma_start(out=wo_sb[:], in_=woh[:])

            # ---- persistent activations ----------------------------------
            qT_sb = state.tile([128, T], BF16)   # [2-head dims, tokens]
            kT_sb = state.tile([128, T], BF16)
            # [tok, h0 v(64), ones, pad, h1 v(64), ones, pad]
            vN_sb = state.tile([128, NTB, 132], BF16)
            ctxT_sb = state.tile([128, T], BF16)  # normalized ctx, [dims, tok]

            nc.vector.memset(vN_sb[:, :, 64:65], 1.0)
            nc.vector.memset(vN_sb[:, :, 65:66], 0.0)
            nc.vector.memset(vN_sb[:, :, 130:131], 1.0)
            nc.vector.memset(vN_sb[:, :, 131:132], 0.0)

            # ---- phase A unit: Q/K projections for one 512-token tile ----
            def emit_qk(tt):
                ts = slice(tt * 512, (tt + 1) * 512)
                ps = ps_s.tile([128, 2, 512], F32, tag="s", name="qk_ps")
                for kc in range(NKC):
                    nc.tensor.matmul(ps[:, 0, :], wq_sb[:, kc, :],
                                     x_sl(kc, ts.start, 512),
                                     start=(kc == 0), stop=(kc == NKC - 1))
                for kc in range(NKC):
                    nc.tensor.matmul(ps[:, 1, :], wk_sb[:, kc, :],
                                     x_sl(kc, ts.start, 512),
                                     start=(kc == 0), stop=(kc == NKC - 1))
                nc.vector.tensor_scalar_add(qT_sb[:, ts], ps[:, 0, :],
                                            bq_sb[:])
                nc.vector.tensor_scalar_add(kT_sb[:, ts], ps[:, 1, :],
                                            bk_sb[:])

            # ---- phase A unit: direct-transposed V for 8 token blocks ----
            def emit_v8(tb0):
                ps = ps_s.tile([128, 2, 512], F32, tag="s", name="v_ps")
                for tb8 in range(8):
                    tb = tb0 + tb8
                    c = (tb8 % 4) * 128
                    for kc in range(NKC):
                        nc.tensor.matmul(
                            ps[:, tb8 // 4, c:c + 128],
                            x_sl(kc, tb * 128, 128), wv_sb[:, kc, :],
                            start=(kc == 0 and tb8 % 4 == 0),
                            stop=(kc == NKC - 1 and tb8 % 4 == 3),
                            skip_group_check=True)
                for tb8 in range(8):
                    tb = tb0 + tb8
                    c = (tb8 % 4) * 128
                    nc.vector.tensor_add(vN_sb[:, tb, 0:64],
                                         ps[:, tb8 // 4, c:c + 64],
                                         bv_bc[:, 0:64])
                    nc.vector.tensor_add(vN_sb[:, tb, 66:130],
                                         ps[:, tb8 // 4, c + 64:c + 128],
                                         bv_bc[:, 66:130])

            # ---- attention for one (batch, 512-query group) --------------
            def emit_scores(b, qt, kb):
                t0 = b * S
                q0 = t0 + qt * 512
                c0 = max(kb - 4 * qt, 0) * 128
                ks = slice(t0 + kb * 128, t0 + (kb + 1) * 128)
                s_ps = ps_s.tile([128, 2, 512], F32, tag="s", name="s_ps")
                nc.tensor.matmul(s_ps[:, 0, c0:512],
                                 kT_sb[0:64, ks], qT_sb[0:64, q0 + c0:q0 + 512],
                                 start=True, stop=True)
                nc.tensor.matmul(s_ps[:, 1, c0:512],
                                 kT_sb[64:128, ks],
                                 qT_sb[64:128, q0 + c0:q0 + 512],
                                 start=True, stop=True)
                return s_ps

            def emit_attn_qt(b, qt, fillers):
                t0 = b * S
                q0 = t0 + qt * 512
                nkb = 4 * qt + 4
                ctx_ps = [ps_c.tile([65, 512], F32, tag="c", name=f"ctx{h}")
                          for h in range(2)]
                s_tiles = {0: emit_scores(b, qt, 0)}
                for kb in range(nkb):
                    m = kb - 4 * qt
                    c0 = max(m, 0) * 128
                    if kb + 1 < nkb:
                        s_tiles[kb + 1] = emit_scores(b, qt, kb + 1)
                    s_ps = s_tiles.pop(kb)
                    e_sb = ep.tile([128, 2, 512], BF16, tag="e", name="e_sb")
                    nc.scalar.activation(e_sb[:, :, c0:512],
                                         s_ps[:, :, c0:512],
                                         AFT.Exp, scale=0.125)
                    if m >= 0:  # triangular block on the diagonal
                        for h in range(2):
                            nc.vector.tensor_mul(
                                e_sb[:, h, c0:c0 + 128],
                                e_sb[:, h, c0:c0 + 128], tri_sb[:])
                    for h in range(2):
                        nc.tensor.matmul(
                            ctx_ps[h][0:65, c0:512],
                            vN_sb[:, b * SB + kb, 66 * h:66 * h + 65],
                            e_sb[:, h, c0:512],
                            start=(kb == 0), stop=(kb == nkb - 1),
                            skip_group_check=True)
                    if fillers and kb % 4 == 3:
                        fillers.pop(0)()
                # normalize: ctx rows 0-63, denominator row 64
                for h in range(2):
                    # denominator row -> bf16 -> K=1 broadcast matmul ->
                    # fast reciprocal on 64 lanes -> one multiply.
                    den = rp.tile([1, 512], BF16, tag="r", name="den")
                    with nc.allow_low_precision("bf16 softmax denominator"):
                        nc.vector.tensor_copy(den[:], ctx_ps[h][64:65, :])
                    bc_ps = ps_c.tile([65, 512], F32, tag="c", name="bc_ps")
                    nc.tensor.matmul(bc_ps[:], ones65[:], den[:],
                                     start=True, stop=True)
                    rc_sb = rp.tile([64, 512], F32, tag="bc", name="rc_sb")
                    nc.vector.reciprocal_approx_fast(rc_sb[:], bc_ps[0:64, :])
                    nc.vector.tensor_mul(
                        ctxT_sb[64 * h:64 * h + 64, q0:q0 + 512],
                        ctx_ps[h][0:64, :], rc_sb[:])
                while fillers:  # flush anything not consumed mid-loop
                    fillers.pop(0)()

            # ---- AllToAll + local full-width output projection -----------
            def emit_half_a2a(b, hf):
                base = b * S + hf * (S // 2)
                ctxd = dram.tile([N_CORES, 128, PH], BF16, tag="ctxd",
                                 name="ctxd", bufs=4)
                # one interleaved DMA: (p, j, t) walk on both sides
                nc.sync.dma_start(
                    out=bass.AP(tensor=ctxd.tensor, offset=ctxd.offset,
                                ap=[[PH, 128], [128 * PH, N_CORES], [1, PH]]),
                    in_=bass.AP(tensor=ctxT_sb.tensor,
                                offset=ctxT_sb.offset + base,
                                ap=[[T, 128], [PH, N_CORES], [1, PH]]))
                recv = dram.tile([N_CORES, 128, PH], BF16, tag="recv",
                                 name="recv", bufs=4)
                nc.gpsimd.collective_compute(
                    "AllToAll",
                    mybir.AluOpType.bypass,
                    replica_groups=[list(range(N_CORES))],
                    ins=[ctxd.opt()],
                    outs=[recv.opt()],
                )
                return recv

            def emit_half_proj(b, hf, recv):
                cg_sb = op.tile([128, NKC, PH], BF16, tag="cg", name="cg_sb")
                nc.sync.dma_start(
                    out=cg_sb[:],
                    in_=bass.AP(tensor=recv.tensor, offset=recv.offset,
                                ap=[[PH, 128], [128 * PH, N_CORES], [1, PH]]))
                o_sb = op.tile([PH, E], F32, tag="o", name="o_sb")
                for et in range(2):
                    ps = ps_w.tile([128, 512], F32, tag="w", name="w_ps")
                    for kc in range(NKC):
                        nc.tensor.matmul(
                            ps[0:PH, :],
                            cg_sb[:, kc, :],
                            wo_sb[:, kc, et * 512:(et + 1) * 512],
                            start=(kc == 0), stop=(kc == NKC - 1))
                    nc.vector.tensor_add(
                        o_sb[:, et * 512:(et + 1) * 512], ps[0:PH, :],
                        bo_bc[0:PH, et * 512:(et + 1) * 512])
                r0 = (b * 2 + hf) * PH
                nc.sync.dma_start(out=out[r0:r0 + PH, :], in_=o_sb[:])

            # ---- schedule -------------------------------------------------
            # Phase A for batch 0 up front; batch 1's A-units and the
            # Wo projections slot in as PE fillers between batch-0 qt
            # groups (the attention phase is ACT-bound).
            for tt in range(4):
                emit_qk(tt)
                if tt % 2 == 1:
                    emit_v8((tt - 1) * 4)

            pending = []

            def mk_fill(fn, *a):
                return lambda: fn(*a)

            for b in range(B):
                for qt in range(4):
                    fillers = []
                    if b == 0:
                        tt = 4 + qt
                        fillers.append(mk_fill(emit_qk, tt))
                        if qt % 2 == 1:
                            fillers.append(mk_fill(emit_v8, (tt - 1) * 4))
                    if pending and (qt % 2 == 0):
                        fillers.append(mk_fill(emit_half_proj,
                                               *pending.pop(0)))
                    emit_attn_qt(b, qt, fillers)
                    if qt == 1 or qt == 3:
                        pending.append((b, qt // 2, emit_half_a2a(b, qt // 2)))

            while pending:
                emit_half_proj(*pending.pop(0))

    nc.compile()
    return nc


_NC = None


def _get_program():
    global _NC
    if _NC is None:
        _NC = build_program()
    return _NC


def _bf(a):
    return np.ascontiguousarray(a).astype(ml_dtypes.bfloat16)


def _pkc(w):
    """[E, d] -> [p(128), kc(8), d] host layout."""
    d = w.shape[1]
    return np.ascontiguousarray(
        w.reshape(NKC, 128, d).transpose(1, 0, 2))


def kernel(x, Wq, bq, Wk, bk, Wv, bv, Wo, bo, _trace=False, _trace_kwargs=None):
    x = np.asarray(x, np.float32)
    Wq, Wk, Wv, Wo = (np.asarray(w, np.float32) for w in (Wq, Wk, Wv, Wo))
    bq, bk, bv, bo = (np.asarray(v, np.float32) for v in (bq, bk, bv, bo))

    xh = _bf(_pkc(x.reshape(T, E).T))
    i = np.arange(128)
    tri = _bf((i[:, None] <= i[None, :]).astype(np.float32))
    woh = _bf(_pkc(Wo.T))

    in_maps = []
    for c in range(N_CORES):
        sl = slice(c * DPC, (c + 1) * DPC)
        bvv = np.zeros(132, np.float32)
        bvv[0:64] = bv[sl][0:64]
        bvv[66:130] = bv[sl][64:128]
        in_maps.append({
            "xh": xh,
            "wqh": _bf(_pkc(Wq[sl, :].T)),
            "wkh": _bf(_pkc(Wk[sl, :].T)),
            "wvh": _bf(_pkc(Wv[sl, :].T)),
            "woh": woh,
            "bq": bq[sl].reshape(DPC, 1).copy(),
            "bk": bk[sl].reshape(DPC, 1).copy(),
            "bvv": bvv,
            "bo": bo,
            "tri": tri,
        })

    nc = _get_program()
    res = run_bass_kernel_spmd(nc, in_maps, list(range(N_CORES)),
                               trace=_trace, **(_trace_kwargs or {}))
    # out[c] rows are [batch, half, 128]: row (b, hf, r) holds global
    # token b*2048 + hf*1024 + c*128 + r.
    stacked = np.stack([res.results[i]["out"].reshape(B, 2, 128, E)
                        for i in range(N_CORES)], axis=2)
    full = stacked.reshape(T, E)
    if _trace:
        return full.reshape(B, S, E), res
    return full.reshape(B, S, E)


# revision 16
# speedup vs baseline: 1.1825x; 1.0966x over previous
"""Multi-head attention (B=2, S=2048, H=16, D=64) on 8 Trainium2 NeuronCores.

Sharding: head-parallel tensor parallelism. Core c owns heads {2c, 2c+1}
(a 128-dim slice of the model dim): column-parallel QKV projections and
local causal attention for its 2 heads, then AllToAlls of normalized
bf16 context vectors pipelined behind the attention loop; each core runs
the full-width Wo projection for its own disjoint token slices and
writes final output rows directly.

Structure (v4, shaped by trace analysis of earlier revisions):
- x streams over the sync HWDGE queue in eight 1 MB chunks whose host
  layout is contiguous per partition (128 8KB-run descriptors each);
  weights and constants ride the scalar HWDGE queue in parallel, so the
  first projection matmul issues ~13 us in.
- All projection work (Q/K per 512 tokens, direct-transposed V per 4
  token blocks) forms a unit stream consumed one unit per key-block
  inside the attention loops, keeping the PE fed while the scalar
  engine (exp, the attention-phase bottleneck) churns; this also keeps
  PE-idle gaps under the ~3.4 us HAM window so matmuls stay at 2.4 GHz.
- Scores run as two concurrent K=64 matmuls (head 0 on PE rows 0-63,
  head 1 on rows 64-127 via tile_position row tiling) into adjacent
  PSUM banks; one Exp activation covers both heads through a strided
  PSUM access pattern.
- Attention-times-V keeps V as the stationary operand ([keys, 64+ones])
  and streams the exp tile, so the context lands directly in
  [dims, tokens] layout -- no PE transposes anywhere.
- The softmax denominator rides a ones-column in the V stationary; it
  is broadcast across partitions with a K=1 matmul, inverted with the
  fast custom-DVE reciprocal, and folded in with one multiply. The
  first two score pairs of the next query group are hoisted above the
  normalization chain to cover its latency.
- AllToAlls go per batch in three token groups (1024/512/512) so the
  final exposed collective moves only 128 KB; output projections lag
  their collective by two groups.
"""

import sys

sys.path.insert(0, "/opt/trn_rl_repo")

import ml_dtypes
import numpy as np

import concourse.bass as bass
import concourse.tile as tile
from concourse import bacc, mybir
from concourse.bass_utils import run_bass_kernel_spmd

N_CORES = 8
B, S, H, D = 2, 2048, 16, 64
E = H * D            # 1024
T = B * S            # 4096 tokens
DPC = 128            # dims (2 heads) per core
NKC = E // 128       # 8 contraction chunks for the projections
NTT = T // 512       # 8 token tiles of 512
NTB = T // 128       # 32 token blocks of 128
SB = S // 128        # 16 key blocks per batch

F32 = mybir.dt.float32
BF16 = mybir.dt.bfloat16
AFT = mybir.ActivationFunctionType

# a2a groups per batch: (token base within batch, tokens per core)
A2A_GROUPS = [(0, 128), (1024, 64), (1536, 64)]
# output rows per core per batch block: [0:128]=G0, [128:192]=G1, [192:256]=G2
OUT_OFF = [0, 128, 192]


def build_program():
    nc = bacc.Bacc("TRN2", target_bir_lowering=False, debug=False,
                   num_devices=N_CORES)

    # host pre-arranged layouts (see kernel()):
    #   xh[p, tt, kc, tl] = x^T[kc*128+p, tt*512+tl]   (bf16)
    #   w*h[p, kc, d]     = W*^T[kc*128+p, d]          (bf16)
    xh = nc.dram_tensor("xh", [128, NTT, NKC, 512], BF16,
                        kind="ExternalInput").ap()
    wqh = nc.dram_tensor("wqh", [128, NKC, DPC], BF16, kind="ExternalInput").ap()
    wkh = nc.dram_tensor("wkh", [128, NKC, DPC], BF16, kind="ExternalInput").ap()
    wvh = nc.dram_tensor("wvh", [128, NKC, DPC], BF16, kind="ExternalInput").ap()
    woh = nc.dram_tensor("woh", [128, NKC, E], BF16, kind="ExternalInput").ap()
    bq = nc.dram_tensor("bq", [DPC, 1], F32, kind="ExternalInput").ap()
    bk = nc.dram_tensor("bk", [DPC, 1], F32, kind="ExternalInput").ap()
    bvv = nc.dram_tensor("bvv", [132], F32, kind="ExternalInput").ap()
    bo = nc.dram_tensor("bo", [E], F32, kind="ExternalInput").ap()
    # [128, 128] lower-triangular keep-mask (k_local <= q_local)
    tri = nc.dram_tensor("tri", [128, 128], BF16, kind="ExternalInput").ap()
    out = nc.dram_tensor("out", [T // N_CORES, E], F32, kind="ExternalOutput").ap()

    with tile.TileContext(nc) as tc:
        with (
            tc.tile_pool(name="consts", bufs=1) as consts,
            tc.tile_pool(name="state", bufs=1) as state,
            tc.tile_pool(name="ep", bufs=3) as ep,
            tc.tile_pool(name="rp", bufs=2) as rp,
            tc.tile_pool(name="op", bufs=2) as op,
            tc.tile_pool(name="ps_s", bufs=2, space="PSUM") as ps_s,
            tc.tile_pool(name="ps_c", bufs=3, space="PSUM") as ps_c,
            tc.tile_pool(name="ps_w", bufs=1, space="PSUM") as ps_w,
            tc.tile_pool(name="dram", bufs=1, space="DRAM") as dram,
        ):
            # ---- weights + consts on the scalar HWDGE queue ---------------
            wq_sb = consts.tile([128, NKC, DPC], BF16)
            wk_sb = consts.tile([128, NKC, DPC], BF16)
            wv_sb = consts.tile([128, NKC, DPC], BF16)
            nc.scalar.dma_start(out=wq_sb[:], in_=wqh[:])
            nc.scalar.dma_start(out=wk_sb[:], in_=wkh[:])
            nc.scalar.dma_start(out=wv_sb[:], in_=wvh[:])
            bq_sb = consts.tile([128, 1], F32)
            bk_sb = consts.tile([128, 1], F32)
            nc.scalar.dma_start(out=bq_sb[:], in_=bq[:])
            nc.scalar.dma_start(out=bk_sb[:], in_=bk[:])
            bv_bc = consts.tile([128, 132], F32)
            nc.scalar.dma_start(
                out=bv_bc[:],
                in_=bass.AP(tensor=bvv.tensor, offset=bvv.offset,
                            ap=[[0, 128], [1, 132]]))
            bo_bc = consts.tile([128, E], F32)
            nc.scalar.dma_start(
                out=bo_bc[:],
                in_=bass.AP(tensor=bo.tensor, offset=bo.offset,
                            ap=[[0, 128], [1, E]]))
            tri_sb = consts.tile([128, 128], BF16)
            nc.scalar.dma_start(out=tri_sb[:], in_=tri[:])
            wo_sb = consts.tile([128, NKC, E], BF16)
            nc.scalar.dma_start(out=wo_sb[:], in_=woh[:])
            ones65 = consts.tile([1, 65], BF16)
            nc.vector.memset(ones65[:], 1.0)

            # ---- x: eight 1 MB contiguous-per-partition chunks (sync) -----
            x_c = [state.tile([128, NKC, 512], BF16, name=f"x_c{i}")
                   for i in range(NTT)]
            for tt in range(NTT):
                nc.sync.dma_start(out=x_c[tt][:], in_=xh[:, tt, :, :])

            def x_sl(kc, t0_, n):
                return x_c[t0_ // 512][:, kc, t0_ % 512:t0_ % 512 + n]

            # ---- persistent activations ----------------------------------
            qT_sb = state.tile([128, T], BF16)   # [2-head dims, tokens]
            kT_sb = state.tile([128, T], BF16)
            # [tok, h0 v(64), ones, pad, h1 v(64), ones, pad]
            vN_sb = state.tile([128, NTB, 132], BF16)
            ctxT_sb = state.tile([128, T], BF16)  # normalized ctx, [dims, tok]

            nc.vector.memset(vN_sb[:, :, 64:65], 1.0)
            nc.vector.memset(vN_sb[:, :, 65:66], 0.0)
            nc.vector.memset(vN_sb[:, :, 130:131], 1.0)
            nc.vector.memset(vN_sb[:, :, 131:132], 0.0)

            # ---- phase A unit: Q/K projections for one 512-token tile ----
            def emit_qk(tt):
                ts = slice(tt * 512, (tt + 1) * 512)
                ps = ps_s.tile([128, 2, 512], F32, tag="s", name="qk_ps")
                for kc in range(NKC):
                    nc.tensor.matmul(ps[:, 0, :], wq_sb[:, kc, :],
                                     x_sl(kc, ts.start, 512),
                                     start=(kc == 0), stop=(kc == NKC - 1))
                for kc in range(NKC):
                    nc.tensor.matmul(ps[:, 1, :], wk_sb[:, kc, :],
                                     x_sl(kc, ts.start, 512),
                                     start=(kc == 0), stop=(kc == NKC - 1))
                nc.vector.tensor_scalar_add(qT_sb[:, ts], ps[:, 0, :],
                                            bq_sb[:])
                nc.vector.tensor_scalar_add(kT_sb[:, ts], ps[:, 1, :],
                                            bk_sb[:])

            # ---- phase A unit: direct-transposed V for 4 token blocks ----
            def emit_v4(tb0):
                ps = ps_s.tile([128, 2, 512], F32, tag="s", name="v_ps")
                for tb4 in range(4):
                    tb = tb0 + tb4
                    c = tb4 * 128
                    for kc in range(NKC):
                        nc.tensor.matmul(
                            ps[:, 0, c:c + 128],
                            x_sl(kc, tb * 128, 128), wv_sb[:, kc, :],
                            start=(kc == 0 and tb4 == 0),
                            stop=(kc == NKC - 1 and tb4 == 3),
                            skip_group_check=True)
                for tb4 in range(4):
                    tb = tb0 + tb4
                    c = tb4 * 128
                    nc.vector.tensor_add(vN_sb[:, tb, 0:64],
                                         ps[:, 0, c:c + 64], bv_bc[:, 0:64])
                    nc.vector.tensor_add(vN_sb[:, tb, 66:130],
                                         ps[:, 0, c + 64:c + 128],
                                         bv_bc[:, 66:130])

            # ---- attention for one (batch, 512-query group) --------------
            def emit_scores(b, qt, kb):
                t0 = b * S
                q0 = t0 + qt * 512
                c0 = max(kb - 4 * qt, 0) * 128
                ks = slice(t0 + kb * 128, t0 + (kb + 1) * 128)
                s_ps = ps_s.tile([128, 2, 512], F32, tag="s", name="s_ps")
                nc.tensor.matmul(s_ps[:, 0, c0:512],
                                 kT_sb[0:64, ks], qT_sb[0:64, q0 + c0:q0 + 512],
                                 start=True, stop=True)
                nc.tensor.matmul(s_ps[:, 1, c0:512],
                                 kT_sb[64:128, ks],
                                 qT_sb[64:128, q0 + c0:q0 + 512],
                                 start=True, stop=True)
                return s_ps

            def emit_attn_qt(b, qt, fillers, pre=None, nxt=None):
                t0 = b * S
                q0 = t0 + qt * 512
                nkb = 4 * qt + 4
                ctx_ps = [ps_c.tile([65, 512], F32, tag="c", name=f"ctx{h}")
                          for h in range(2)]
                s_tiles = pre if pre else {0: emit_scores(b, qt, 0)}
                for kb in range(nkb):
                    m = kb - 4 * qt
                    c0 = max(m, 0) * 128
                    if kb + 1 < nkb and kb + 1 not in s_tiles:
                        s_tiles[kb + 1] = emit_scores(b, qt, kb + 1)
                    s_ps = s_tiles.pop(kb)
                    e_sb = ep.tile([128, 2, 512], BF16, tag="e", name="e_sb")
                    nc.scalar.activation(e_sb[:, :, c0:512],
                                         s_ps[:, :, c0:512],
                                         AFT.Exp, scale=0.125)
                    if m >= 0:  # triangular block on the diagonal
                        for h in range(2):
                            nc.vector.tensor_mul(
                                e_sb[:, h, c0:c0 + 128],
                                e_sb[:, h, c0:c0 + 128], tri_sb[:])
                    for h in range(2):
                        nc.tensor.matmul(
                            ctx_ps[h][0:65, c0:512],
                            vN_sb[:, b * SB + kb, 66 * h:66 * h + 65],
                            e_sb[:, h, c0:512],
                            start=(kb == 0), stop=(kb == nkb - 1),
                            skip_group_check=True)
                    if fillers:
                        fillers.popleft()()
                # hoist the next group's first two score pairs above the
                # normalization chain so the PE isn't parked on it
                hoisted = None
                if nxt is not None:
                    hoisted = {0: emit_scores(nxt[0], nxt[1], 0),
                               1: emit_scores(nxt[0], nxt[1], 1)}
                # normalize: ctx rows 0-63, denominator row 64
                for h in range(2):
                    den = rp.tile([1, 512], BF16, tag="r", name="den")
                    with nc.allow_low_precision("bf16 softmax denominator"):
                        nc.vector.tensor_copy(den[:], ctx_ps[h][64:65, :])
                    bc_ps = ps_c.tile([65, 512], F32, tag="c", name="bc_ps")
                    nc.tensor.matmul(bc_ps[:], ones65[:], den[:],
                                     start=True, stop=True)
                    rc_sb = rp.tile([64, 512], F32, tag="bc", name="rc_sb")
                    nc.vector.reciprocal_approx_fast(rc_sb[:], bc_ps[0:64, :])
                    nc.vector.tensor_mul(
                        ctxT_sb[64 * h:64 * h + 64, q0:q0 + 512],
                        ctx_ps[h][0:64, :], rc_sb[:])
                return hoisted

            # ---- AllToAll + local full-width output projection -----------
            def emit_a2a(b, g):
                base, phg = b * S + A2A_GROUPS[g][0], A2A_GROUPS[g][1]
                ctxd = dram.tile([N_CORES, 128, phg], BF16, tag=f"ctxd{phg}",
                                 name="ctxd", bufs=2)
                nc.sync.dma_start(
                    out=bass.AP(tensor=ctxd.tensor, offset=ctxd.offset,
                                ap=[[phg, 128], [128 * phg, N_CORES],
                                    [1, phg]]),
                    in_=bass.AP(tensor=ctxT_sb.tensor,
                                offset=ctxT_sb.offset + base,
                                ap=[[T, 128], [phg, N_CORES], [1, phg]]))
                recv = dram.tile([N_CORES, 128, phg], BF16, tag=f"recv{phg}",
                                 name="recv", bufs=2)
                nc.gpsimd.collective_compute(
                    "AllToAll",
                    mybir.AluOpType.bypass,
                    replica_groups=[list(range(N_CORES))],
                    ins=[ctxd.opt()],
                    outs=[recv.opt()],
                )
                return recv

            def emit_proj(b, g, recv):
                phg = A2A_GROUPS[g][1]
                cg_sb = op.tile([128, NKC, phg], BF16, tag=f"cg{phg}",
                                name="cg_sb")
                nc.sync.dma_start(
                    out=cg_sb[:],
                    in_=bass.AP(tensor=recv.tensor, offset=recv.offset,
                                ap=[[phg, 128], [128 * phg, N_CORES],
                                    [1, phg]]))
                o_sb = op.tile([phg, E], F32, tag=f"o{phg}", name="o_sb")
                for et in range(2):
                    ps = ps_w.tile([128, 512], F32, tag="w", name="w_ps")
                    for kc in range(NKC):
                        nc.tensor.matmul(
                            ps[0:phg, :],
                            cg_sb[:, kc, :],
                            wo_sb[:, kc, et * 512:(et + 1) * 512],
                            start=(kc == 0), stop=(kc == NKC - 1))
                    nc.vector.tensor_add(
                        o_sb[:, et * 512:(et + 1) * 512], ps[0:phg, :],
                        bo_bc[0:phg, et * 512:(et + 1) * 512])
                r0 = b * 256 + OUT_OFF[g]
                nc.sync.dma_start(out=out[r0:r0 + phg, :], in_=o_sb[:])

            # ---- schedule -------------------------------------------------
            from collections import deque

            # projection unit stream, consumed one unit per key block
            # inside the attention loops (dependencies: attn (b,qt) needs
            # qk(tt<=4b+qt) and v4 up to tb=4(4b+qt)+3, all of which land
            # earlier in this stream than they are needed)
            units = deque()
            for tt in range(1, NTT):
                units.append(lambda t=tt: emit_qk(t))
                if tt + 1 < NTT:
                    units.append(lambda t=tt: emit_v4(4 * (t + 1)))

            emit_qk(0)
            emit_v4(0)
            emit_v4(4)

            groups = [(b, qt) for b in range(B) for qt in range(4)]
            pending = []   # (group_idx_emitted, b, g, recv)
            pre = None
            for gi, (b, qt) in enumerate(groups):
                fillers = deque()
                # matured output projections (a2a issued >= 2 groups ago)
                still = []
                for (egi, pb, pg, recv) in pending:
                    if gi - egi >= 2:
                        fillers.append(
                            lambda a=pb, c=pg, r=recv: emit_proj(a, c, r))
                    else:
                        still.append((egi, pb, pg, recv))
                pending = still
                for _ in range(3):  # up to 3 projection units per group
                    if units:
                        fillers.append(units.popleft())
                nxt = groups[gi + 1] if gi + 1 < len(groups) else None
                pre = emit_attn_qt(b, qt, fillers, pre=pre, nxt=nxt)
                while fillers:
                    fillers.popleft()()
                if qt >= 1:  # a2a for group qt-1 boundaries: qt1->G0 etc.
                    g = qt - 1
                    pending.append((gi, b, g, emit_a2a(b, g)))

            while units:
                units.popleft()()
            for (egi, pb, pg, recv) in pending:
                emit_proj(pb, pg, recv)

    nc.compile()
    return nc


_NC = None


def _get_program():
    global _NC
    if _NC is None:
        _NC = build_program()
    return _NC


def _bf(a):
    return np.ascontiguousarray(a).astype(ml_dtypes.bfloat16)


def _pkc(w):
    """[E, d] -> [p(128), kc(8), d] host layout."""
    d = w.shape[1]
    return np.ascontiguousarray(
        w.reshape(NKC, 128, d).transpose(1, 0, 2))


def kernel(x, Wq, bq, Wk, bk, Wv, bv, Wo, bo, _trace=False, _trace_kwargs=None):
    x = np.asarray(x, np.float32)
    Wq, Wk, Wv, Wo = (np.asarray(w, np.float32) for w in (Wq, Wk, Wv, Wo))
    bq, bk, bv, bo = (np.asarray(v, np.float32) for v in (bq, bk, bv, bo))

    # xh[p, tt, kc, tl] = x^T[kc*128+p, tt*512+tl]
    xh = _bf(x.reshape(T, E).T.reshape(NKC, 128, NTT, 512)
             .transpose(1, 2, 0, 3))
    i = np.arange(128)
    tri = _bf((i[:, None] <= i[None, :]).astype(np.float32))
    woh = _bf(_pkc(Wo.T))

    in_maps = []
    for c in range(N_CORES):
        sl = slice(c * DPC, (c + 1) * DPC)
        bvv = np.zeros(132, np.float32)
        bvv[0:64] = bv[sl][0:64]
        bvv[66:130] = bv[sl][64:128]
        in_maps.append({
            "xh": xh,
            "wqh": _bf(_pkc(Wq[sl, :].T)),
            "wkh": _bf(_pkc(Wk[sl, :].T)),
            "wvh": _bf(_pkc(Wv[sl, :].T)),
            "woh": woh,
            "bq": bq[sl].reshape(DPC, 1).copy(),
            "bk": bk[sl].reshape(DPC, 1).copy(),
            "bvv": bvv,
            "bo": bo,
            "tri": tri,
        })

    nc = _get_program()
    res = run_bass_kernel_spmd(nc, in_maps, list(range(N_CORES)),
                               trace=_trace, **(_trace_kwargs or {}))
    # per-core out rows: per batch block of 256 rows:
    #   [0:128]   = tokens b*2048 +    0 + c*128 + r   (group 0)
    #   [128:192] = tokens b*2048 + 1024 + c*64  + r   (group 1)
    #   [192:256] = tokens b*2048 + 1536 + c*64  + r   (group 2)
    full = np.empty((T, E), np.float32)
    for c in range(N_CORES):
        o = np.asarray(res.results[c]["out"])
        for b in range(B):
            t0 = b * S
            full[t0 + c * 128:t0 + c * 128 + 128] = o[b * 256:b * 256 + 128]
            full[t0 + 1024 + c * 64:t0 + 1024 + c * 64 + 64] = \
                o[b * 256 + 128:b * 256 + 192]
            full[t0 + 1536 + c * 64:t0 + 1536 + c * 64 + 64] = \
                o[b * 256 + 192:b * 256 + 256]
    if _trace:
        return full.reshape(B, S, E), res
    return full.reshape(B, S, E)


# revision 19
# speedup vs baseline: 1.2713x; 1.0751x over previous
"""Multi-head attention (B=2, S=2048, H=16, D=64) on 8 Trainium2 NeuronCores.

Sharding: head-parallel tensor parallelism. Core c owns heads {2c, 2c+1}
(a 128-dim slice of the model dim): column-parallel QKV projections and
local causal attention for its 2 heads, then AllToAlls of normalized
bf16 context vectors pipelined behind the attention loop; each core runs
the full-width Wo projection for its own disjoint token slices and
writes final output rows directly.

Structure (v4, shaped by trace analysis of earlier revisions):
- x streams over the sync HWDGE queue in eight 1 MB chunks whose host
  layout is contiguous per partition (128 8KB-run descriptors each);
  weights and constants ride the scalar HWDGE queue in parallel, so the
  first projection matmul issues ~13 us in.
- All projection work (Q/K per 512 tokens, direct-transposed V per 4
  token blocks) forms a unit stream consumed one unit per key-block
  inside the attention loops, keeping the PE fed while the scalar
  engine (exp, the attention-phase bottleneck) churns; this also keeps
  PE-idle gaps under the ~3.4 us HAM window so matmuls stay at 2.4 GHz.
- Scores run as two concurrent K=64 matmuls (head 0 on PE rows 0-63,
  head 1 on rows 64-127 via tile_position row tiling) into adjacent
  PSUM banks; one Exp activation covers both heads through a strided
  PSUM access pattern.
- Attention-times-V keeps V as the stationary operand ([keys, 64+ones])
  and streams the exp tile, so the context lands directly in
  [dims, tokens] layout -- no PE transposes anywhere.
- The softmax denominator rides a ones-column in the V stationary; it
  is broadcast across partitions with a K=1 matmul, inverted with the
  fast custom-DVE reciprocal, and folded in with one multiply. The
  first two score pairs of the next query group are hoisted above the
  normalization chain to cover its latency.
- AllToAlls go per batch in three token groups (1024/512/512) so the
  final exposed collective moves only 128 KB; output projections lag
  their collective by two groups.
"""

import sys

sys.path.insert(0, "/opt/trn_rl_repo")

import ml_dtypes
import numpy as np

import concourse.bass as bass
import concourse.tile as tile
from concourse import bacc, mybir
from concourse.bass_utils import run_bass_kernel_spmd

N_CORES = 8
B, S, H, D = 2, 2048, 16, 64
E = H * D            # 1024
T = B * S            # 4096 tokens
DPC = 128            # dims (2 heads) per core
NKC = E // 128       # 8 contraction chunks for the projections
NTT = T // 512       # 8 token tiles of 512
NTB = T // 128       # 32 token blocks of 128
SB = S // 128        # 16 key blocks per batch

F32 = mybir.dt.float32
BF16 = mybir.dt.bfloat16
AFT = mybir.ActivationFunctionType

# a2a groups per batch: (token base within batch, tokens per core)
A2A_GROUPS = [(0, 128), (1024, 64), (1536, 64)]
# output rows per core per batch block: [0:128]=G0, [128:192]=G1, [192:256]=G2
OUT_OFF = [0, 128, 192]


def build_program():
    nc = bacc.Bacc("TRN2", target_bir_lowering=False, debug=False,
                   num_devices=N_CORES)

    # host pre-arranged layouts (see kernel()):
    #   xh[p, tt, kc, tl] = x^T[kc*128+p, tt*512+tl]   (bf16)
    #   w*h[p, kc, d]     = W*^T[kc*128+p, d]          (bf16)
    xh = nc.dram_tensor("xh", [128, NTT, NKC, 512], BF16,
                        kind="ExternalInput").ap()
    wqh = nc.dram_tensor("wqh", [128, NKC, DPC], BF16, kind="ExternalInput").ap()
    wkh = nc.dram_tensor("wkh", [128, NKC, DPC], BF16, kind="ExternalInput").ap()
    wvh = nc.dram_tensor("wvh", [128, NKC, DPC], BF16, kind="ExternalInput").ap()
    woh = nc.dram_tensor("woh", [128, NKC, E], BF16, kind="ExternalInput").ap()
    bq = nc.dram_tensor("bq", [DPC, 1], F32, kind="ExternalInput").ap()
    bk = nc.dram_tensor("bk", [DPC, 1], F32, kind="ExternalInput").ap()
    bvv = nc.dram_tensor("bvv", [132], F32, kind="ExternalInput").ap()
    bo = nc.dram_tensor("bo", [E], F32, kind="ExternalInput").ap()
    # [128, 128] lower-triangular keep-mask (k_local <= q_local)
    tri = nc.dram_tensor("tri", [128, 128], BF16, kind="ExternalInput").ap()
    out = nc.dram_tensor("out", [T // N_CORES, E], F32, kind="ExternalOutput").ap()

    with tile.TileContext(nc) as tc:
        with (
            tc.tile_pool(name="consts", bufs=1) as consts,
            tc.tile_pool(name="state", bufs=1) as state,
            tc.tile_pool(name="ep", bufs=3) as ep,
            tc.tile_pool(name="rp", bufs=2) as rp,
            tc.tile_pool(name="op", bufs=2) as op,
            tc.tile_pool(name="ps_s", bufs=2, space="PSUM") as ps_s,
            tc.tile_pool(name="ps_c", bufs=3, space="PSUM") as ps_c,
            tc.tile_pool(name="ps_w", bufs=1, space="PSUM") as ps_w,
            tc.tile_pool(name="dram", bufs=1, space="DRAM") as dram,
        ):
            # ---- weights + consts on the scalar HWDGE queue ---------------
            wq_sb = consts.tile([128, NKC, DPC], BF16)
            wk_sb = consts.tile([128, NKC, DPC], BF16)
            wv_sb = consts.tile([128, NKC, DPC], BF16)
            nc.scalar.dma_start(out=wq_sb[:], in_=wqh[:])
            nc.scalar.dma_start(out=wk_sb[:], in_=wkh[:])
            nc.scalar.dma_start(out=wv_sb[:], in_=wvh[:])
            bq_sb = consts.tile([128, 1], F32)
            bk_sb = consts.tile([128, 1], F32)
            nc.scalar.dma_start(out=bq_sb[:], in_=bq[:])
            nc.scalar.dma_start(out=bk_sb[:], in_=bk[:])
            bv_bc = consts.tile([128, 132], F32)
            nc.scalar.dma_start(
                out=bv_bc[:],
                in_=bass.AP(tensor=bvv.tensor, offset=bvv.offset,
                            ap=[[0, 128], [1, 132]]))
            bo_bc = consts.tile([128, E], F32)
            nc.scalar.dma_start(
                out=bo_bc[:],
                in_=bass.AP(tensor=bo.tensor, offset=bo.offset,
                            ap=[[0, 128], [1, E]]))
            tri_sb = consts.tile([128, 128], BF16)
            nc.scalar.dma_start(out=tri_sb[:], in_=tri[:])
            wo_sb = consts.tile([128, NKC, E], BF16)
            nc.scalar.dma_start(out=wo_sb[:], in_=woh[:])
            ones65 = consts.tile([128, 65], BF16)
            nc.vector.memset(ones65[:], 1.0)

            # ---- x: eight 1 MB contiguous-per-partition chunks (sync) -----
            x_c = [state.tile([128, NKC, 512], BF16, name=f"x_c{i}")
                   for i in range(NTT)]
            for tt in range(NTT):
                nc.sync.dma_start(out=x_c[tt][:], in_=xh[:, tt, :, :])

            def x_sl(kc, t0_, n):
                return x_c[t0_ // 512][:, kc, t0_ % 512:t0_ % 512 + n]

            # ---- persistent activations ----------------------------------
            qT_sb = state.tile([128, T], BF16)   # [2-head dims, tokens]
            kT_sb = state.tile([128, T], BF16)
            # [tok, h0 v(64), ones, pad, h1 v(64), ones, pad]
            vN_sb = state.tile([128, NTB, 132], BF16)
            ctxT_sb = state.tile([128, T], BF16)  # normalized ctx, [dims, tok]

            nc.vector.memset(vN_sb[:, :, 64:65], 1.0)
            nc.vector.memset(vN_sb[:, :, 65:66], 0.0)
            nc.vector.memset(vN_sb[:, :, 130:131], 1.0)
            nc.vector.memset(vN_sb[:, :, 131:132], 0.0)

            # ---- phase A unit: Q/K projections for one 512-token tile ----
            def emit_qk(tt):
                ts = slice(tt * 512, (tt + 1) * 512)
                ps = ps_s.tile([128, 2, 512], F32, tag="s", name="qk_ps")
                for kc in range(NKC):
                    nc.tensor.matmul(ps[:, 0, :], wq_sb[:, kc, :],
                                     x_sl(kc, ts.start, 512),
                                     start=(kc == 0), stop=(kc == NKC - 1))
                for kc in range(NKC):
                    nc.tensor.matmul(ps[:, 1, :], wk_sb[:, kc, :],
                                     x_sl(kc, ts.start, 512),
                                     start=(kc == 0), stop=(kc == NKC - 1))
                nc.vector.tensor_scalar_add(qT_sb[:, ts], ps[:, 0, :],
                                            bq_sb[:])
                nc.vector.tensor_scalar_add(kT_sb[:, ts], ps[:, 1, :],
                                            bk_sb[:])

            # ---- phase A unit: direct-transposed V for 4 token blocks ----
            def emit_v4(tb0):
                ps = ps_s.tile([128, 2, 512], F32, tag="s", name="v_ps")
                for tb4 in range(4):
                    tb = tb0 + tb4
                    c = tb4 * 128
                    for kc in range(NKC):
                        nc.tensor.matmul(
                            ps[:, 0, c:c + 128],
                            x_sl(kc, tb * 128, 128), wv_sb[:, kc, :],
                            start=(kc == 0 and tb4 == 0),
                            stop=(kc == NKC - 1 and tb4 == 3),
                            skip_group_check=True)
                for tb4 in range(4):
                    tb = tb0 + tb4
                    c = tb4 * 128
                    nc.vector.tensor_add(vN_sb[:, tb, 0:64],
                                         ps[:, 0, c:c + 64], bv_bc[:, 0:64])
                    nc.vector.tensor_add(vN_sb[:, tb, 66:130],
                                         ps[:, 0, c + 64:c + 128],
                                         bv_bc[:, 66:130])

            # ---- attention for one (batch, 512-query group) --------------
            def emit_scores(b, qt, kb):
                t0 = b * S
                q0 = t0 + qt * 512
                c0 = max(kb - 4 * qt, 0) * 128
                ks = slice(t0 + kb * 128, t0 + (kb + 1) * 128)
                s_ps = ps_s.tile([128, 2, 512], F32, tag="s", name="s_ps")
                nc.tensor.matmul(s_ps[:, 0, c0:512],
                                 kT_sb[0:64, ks], qT_sb[0:64, q0 + c0:q0 + 512],
                                 start=True, stop=True)
                nc.tensor.matmul(s_ps[:, 1, c0:512],
                                 kT_sb[64:128, ks],
                                 qT_sb[64:128, q0 + c0:q0 + 512],
                                 start=True, stop=True)
                return s_ps

            def emit_attn_qt(b, qt, fillers, pre=None, nxt=None):
                t0 = b * S
                q0 = t0 + qt * 512
                nkb = 4 * qt + 4
                ctx_ps = [ps_c.tile([65, 512], F32, tag="c", name=f"ctx{h}")
                          for h in range(2)]
                s_tiles = pre if pre else {0: emit_scores(b, qt, 0)}
                for kb in range(nkb):
                    m = kb - 4 * qt
                    c0 = max(m, 0) * 128
                    if kb + 1 < nkb and kb + 1 not in s_tiles:
                        s_tiles[kb + 1] = emit_scores(b, qt, kb + 1)
                    s_ps = s_tiles.pop(kb)
                    e_sb = ep.tile([128, 2, 512], BF16, tag="e", name="e_sb")
                    nc.scalar.activation(e_sb[:, :, c0:512],
                                         s_ps[:, :, c0:512],
                                         AFT.Exp, scale=0.125)
                    if m >= 0:  # triangular block on the diagonal
                        for h in range(2):
                            nc.vector.tensor_mul(
                                e_sb[:, h, c0:c0 + 128],
                                e_sb[:, h, c0:c0 + 128], tri_sb[:])
                    for h in range(2):
                        nc.tensor.matmul(
                            ctx_ps[h][0:65, c0:512],
                            vN_sb[:, b * SB + kb, 66 * h:66 * h + 65],
                            e_sb[:, h, c0:512],
                            start=(kb == 0), stop=(kb == nkb - 1),
                            skip_group_check=True)
                    if fillers:
                        fillers.popleft()()
                # evacuate ctx to SBUF unnormalized right away (releases the
                # PSUM bank in one short copy; the bf16 denominator row comes
                # along for free) ...
                ctxU = []
                for h in range(2):
                    cu = op.tile([65, 512], BF16, tag="cu", name="cu", bufs=4)
                    with nc.allow_low_precision("bf16 unnormalized ctx"):
                        nc.vector.tensor_copy(cu[:], ctx_ps[h][0:65, :])
                    ctxU.append(cu)
                # ... hoist the next group's first two score pairs ...
                hoisted = None
                if nxt is not None:
                    hoisted = {0: emit_scores(nxt[0], nxt[1], 0),
                               1: emit_scores(nxt[0], nxt[1], 1)}

                # ... and defer the normalization chain (broadcast matmul,
                # fast reciprocal, one multiply) into the next group's loop.
                def mk_norm(h, cu):
                    def run():
                        bc_ps = ps_w.tile([65, 512], F32, tag="w",
                                          name="bc_ps")
                        nc.tensor.matmul(bc_ps[:], ones65[64:65, :],
                                         cu[64:65, :], start=True, stop=True)
                        rc_sb = rp.tile([64, 512], F32, tag="bc", name="rc_sb")
                        nc.vector.reciprocal_approx_fast(rc_sb[:],
                                                         bc_ps[0:64, :])
                        nc.vector.tensor_mul(
                            ctxT_sb[64 * h:64 * h + 64, q0:q0 + 512],
                            cu[0:64, :], rc_sb[:])
                    return run

                return hoisted, [mk_norm(0, ctxU[0]), mk_norm(1, ctxU[1])]

            # ---- AllToAll + local full-width output projection -----------
            def emit_a2a(b, g):
                base, phg = b * S + A2A_GROUPS[g][0], A2A_GROUPS[g][1]
                ctxd = dram.tile([N_CORES, 128, phg], BF16, tag=f"ctxd{phg}",
                                 name="ctxd", bufs=2)
                nc.sync.dma_start(
                    out=bass.AP(tensor=ctxd.tensor, offset=ctxd.offset,
                                ap=[[phg, 128], [128 * phg, N_CORES],
                                    [1, phg]]),
                    in_=bass.AP(tensor=ctxT_sb.tensor,
                                offset=ctxT_sb.offset + base,
                                ap=[[T, 128], [phg, N_CORES], [1, phg]]))
                recv = dram.tile([N_CORES, 128, phg], BF16, tag=f"recv{phg}",
                                 name="recv", bufs=2)
                nc.gpsimd.collective_compute(
                    "AllToAll",
                    mybir.AluOpType.bypass,
                    replica_groups=[list(range(N_CORES))],
                    ins=[ctxd.opt()],
                    outs=[recv.opt()],
                )
                return recv

            def emit_proj(b, g, recv):
                phg = A2A_GROUPS[g][1]
                cg_sb = op.tile([128, NKC, phg], BF16, tag=f"cg{phg}",
                                name="cg_sb")
                nc.sync.dma_start(
                    out=cg_sb[:],
                    in_=bass.AP(tensor=recv.tensor, offset=recv.offset,
                                ap=[[phg, 128], [128 * phg, N_CORES],
                                    [1, phg]]))
                o_sb = op.tile([phg, E], F32, tag=f"o{phg}", name="o_sb")
                for et in range(2):
                    ps = ps_w.tile([128, 512], F32, tag="w", name="w_ps")
                    for kc in range(NKC):
                        nc.tensor.matmul(
                            ps[0:phg, :],
                            cg_sb[:, kc, :],
                            wo_sb[:, kc, et * 512:(et + 1) * 512],
                            start=(kc == 0), stop=(kc == NKC - 1))
                    nc.vector.tensor_add(
                        o_sb[:, et * 512:(et + 1) * 512], ps[0:phg, :],
                        bo_bc[0:phg, et * 512:(et + 1) * 512])
                r0 = b * 256 + OUT_OFF[g]
                nc.sync.dma_start(out=out[r0:r0 + phg, :], in_=o_sb[:])

            # ---- schedule -------------------------------------------------
            from collections import deque

            # Static placement of projection units as PE fillers inside the
            # attention loops; each unit lands at least one group before its
            # consumer needs it, and the late (ACT-heaviest) groups get the
            # matured output projections.
            qk = emit_qk
            v4 = emit_v4
            unit_sched = {
                (0, 0): [lambda: qk(1)],
                (0, 1): [lambda: qk(2), lambda: v4(8)],
                (0, 2): [lambda: qk(3), lambda: v4(12), lambda: qk(4)],
                (0, 3): [lambda: v4(16), lambda: qk(5), lambda: v4(20)],
                (1, 0): [lambda: qk(6), lambda: v4(24)],
                (1, 1): [lambda: qk(7), lambda: v4(28)],
                (1, 2): [],
                (1, 3): [],
            }

            emit_qk(0)
            emit_v4(0)
            emit_v4(4)

            groups = [(b, qt) for b in range(B) for qt in range(4)]
            projs_due = {}  # group idx -> [proj closures]

            def mk_a2a(b_, g_, due_gi):
                def run():
                    recv = emit_a2a(b_, g_)
                    projs_due.setdefault(due_gi, []).append(
                        lambda: emit_proj(b_, g_, recv))
                return run

            pre = None
            carry = []  # norm + a2a closures from the previous group
            for gi, (b, qt) in enumerate(groups):
                fillers = deque(carry + projs_due.pop(gi, [])
                                + unit_sched[(b, qt)])
                carry = []
                nxt = groups[gi + 1] if gi + 1 < len(groups) else None
                pre, norms = emit_attn_qt(b, qt, fillers, pre=pre, nxt=nxt)
                while fillers:
                    fillers.popleft()()
                carry.extend(norms)
                if qt >= 1:
                    # a2a for token group qt-1; it runs inside group gi+1's
                    # loop, so its projection matures at group gi+2
                    carry.append(mk_a2a(b, qt - 1, gi + 2))

            for c in carry:  # tail: last norms + final a2a
                c()
            for gi in sorted(projs_due):
                for p in projs_due[gi]:
                    p()

    nc.compile()
    return nc


_NC = None


def _get_program():
    global _NC
    if _NC is None:
        _NC = build_program()
    return _NC


def _bf(a):
    return np.ascontiguousarray(a).astype(ml_dtypes.bfloat16)


def _pkc(w):
    """[E, d] -> [p(128), kc(8), d] host layout."""
    d = w.shape[1]
    return np.ascontiguousarray(
        w.reshape(NKC, 128, d).transpose(1, 0, 2))


def kernel(x, Wq, bq, Wk, bk, Wv, bv, Wo, bo, _trace=False, _trace_kwargs=None):
    x = np.asarray(x, np.float32)
    Wq, Wk, Wv, Wo = (np.asarray(w, np.float32) for w in (Wq, Wk, Wv, Wo))
    bq, bk, bv, bo = (np.asarray(v, np.float32) for v in (bq, bk, bv, bo))

    # xh[p, tt, kc, tl] = x^T[kc*128+p, tt*512+tl]
    xh = _bf(x.reshape(T, E).T.reshape(NKC, 128, NTT, 512)
             .transpose(1, 2, 0, 3))
    i = np.arange(128)
    tri = _bf((i[:, None] <= i[None, :]).astype(np.float32))
    woh = _bf(_pkc(Wo.T))

    in_maps = []
    for c in range(N_CORES):
        sl = slice(c * DPC, (c + 1) * DPC)
        bvv = np.zeros(132, np.float32)
        bvv[0:64] = bv[sl][0:64]
        bvv[66:130] = bv[sl][64:128]
        in_maps.append({
            "xh": xh,
            "wqh": _bf(_pkc(Wq[sl, :].T)),
            "wkh": _bf(_pkc(Wk[sl, :].T)),
            "wvh": _bf(_pkc(Wv[sl, :].T)),
            "woh": woh,
            "bq": bq[sl].reshape(DPC, 1).copy(),
            "bk": bk[sl].reshape(DPC, 1).copy(),
            "bvv": bvv,
            "bo": bo,
            "tri": tri,
        })

    nc = _get_program()
    res = run_bass_kernel_spmd(nc, in_maps, list(range(N_CORES)),
                               trace=_trace, **(_trace_kwargs or {}))
    # per-core out rows: per batch block of 256 rows:
    #   [0:128]   = tokens b*2048 +    0 + c*128 + r   (group 0)
    #   [128:192] = tokens b*2048 + 1024 + c*64  + r   (group 1)
    #   [192:256] = tokens b*2048 + 1536 + c*64  + r   (group 2)
    full = np.empty((T, E), np.float32)
    for c in range(N_CORES):
        o = np.asarray(res.results[c]["out"])
        for b in range(B):
            t0 = b * S
            full[t0 + c * 128:t0 + c * 128 + 128] = o[b * 256:b * 256 + 128]
            full[t0 + 1024 + c * 64:t0 + 1024 + c * 64 + 64] = \
                o[b * 256 + 128:b * 256 + 192]
            full[t0 + 1536 + c * 64:t0 + 1536 + c * 64 + 64] = \
                o[b * 256 + 192:b * 256 + 256]
    if _trace:
        return full.reshape(B, S, E), res
    return full.reshape(B, S, E)
